# revision 23
# baseline (speedup 1.0000x reference)
"""Trainium2 Bass kernel for an 8-batch image-conditioned decoder layer.

Strategy: pure data-parallel over the batch — core c computes batch element c
end-to-end (embedding gather, causal self-attention, cross-attention over the
image tokens, both layernorms, vocab projection). No collectives.

All matmuls run in bf16 with fp32 PSUM accumulation.  Weights are pre-cast /
pre-tiled on the host into the exact SBUF layouts the TensorEngine consumes
([128 k_inner, k_outer, n]); the vocab projection is streamed from HBM in
512-column chunks.  The embedding gather happens on-device via indirect DMA
from a bf16 copy of the table.
"""

import os
import sys

for _p in ("/opt/trn_rl_repo", "/root/.axon_site/_ro/trn_rl_repo"):
    if os.path.isdir(_p) and _p not in sys.path:
        sys.path.append(_p)

import numpy as np
import ml_dtypes

BF16 = ml_dtypes.bfloat16

# Problem dims (hardcoded per spec)
V, D, DI, S, B, NI = 32000, 1024, 768, 512, 8, 197
EPS = 1e-5
P = 128
ST = S // P          # 4 seq tiles
DT = D // P          # 8 model-dim tiles
DIT = DI // P        # 6 image-dim tiles
NIT = 2              # image tokens: 197 -> 2 partition tiles (128 + 69)
NI_PAD = 256
VP = 32768           # vocab padded to 64 chunks of 512
CN = 1024            # vocab chunk width
NCHUNK = VP // CN    # 32
GRP = 1              # chunks per output strip
NGRP = NCHUNK // GRP
N_CORES = 8
SCALE = 1.0 / float(np.sqrt(np.float32(D)))

_CACHE = {}
LAST_RESULTS = None


def _build_program():
    import concourse.bacc as bacc
    import concourse.bass as bass
    import concourse.mybir as mybir
    from concourse.masks import make_identity
    from concourse.tile import TileContext

    f32 = mybir.dt.float32
    bf16 = mybir.dt.bfloat16
    i32 = mybir.dt.int32
    X = mybir.AxisListType.X
    ALU = mybir.AluOpType
    ACT_F = mybir.ActivationFunctionType

    nc = bacc.Bacc("TRN2", target_bir_lowering=False, debug=False,
                   num_devices=N_CORES)

    # ---- I/O ----
    h_tok = nc.dram_tensor("tok", [S], i32, kind="ExternalInput")
    h_table = nc.dram_tensor("table", [V, D], bf16, kind="ExternalInput")
    h_pos = nc.dram_tensor("pos", [S, D], f32, kind="ExternalInput")
    h_img = nc.dram_tensor("img_t", [P, DIT, NI], bf16, kind="ExternalInput")
    h_wq1 = nc.dram_tensor("wq1", [P, DT, D], bf16, kind="ExternalInput")
    h_wk1 = nc.dram_tensor("wk1", [P, DT, D], bf16, kind="ExternalInput")
    h_wv1 = nc.dram_tensor("wv1", [P, DT, D], bf16, kind="ExternalInput")
    h_wq2 = nc.dram_tensor("wq2", [P, DT, D], bf16, kind="ExternalInput")
    h_wk2 = nc.dram_tensor("wk2", [P, DIT, D], bf16, kind="ExternalInput")
    h_wv2 = nc.dram_tensor("wv2", [P, DIT, D], bf16, kind="ExternalInput")
    h_wp = nc.dram_tensor("wp", [NCHUNK, P, DT, CN], bf16, kind="ExternalInput")
    h_bq1 = nc.dram_tensor("bq1", [P, DT], f32, kind="ExternalInput")
    h_bk1 = nc.dram_tensor("bk1", [P, DT], f32, kind="ExternalInput")
    h_bq2 = nc.dram_tensor("bq2", [P, DT], f32, kind="ExternalInput")
    h_bk2 = nc.dram_tensor("bk2", [P, DT], f32, kind="ExternalInput")
    h_bv1 = nc.dram_tensor("bv1", [D], f32, kind="ExternalInput")
    h_bv2 = nc.dram_tensor("bv2", [D], f32, kind="ExternalInput")
    h_bp = nc.dram_tensor("bp", [VP], bf16, kind="ExternalInput")
    h_g1 = nc.dram_tensor("g1", [D], f32, kind="ExternalInput")
    h_b1 = nc.dram_tensor("b1", [D], f32, kind="ExternalInput")
    h_g2 = nc.dram_tensor("g2", [D], f32, kind="ExternalInput")
    h_b2 = nc.dram_tensor("b2", [D], f32, kind="ExternalInput")
    h_out = nc.dram_tensor("out", [S, VP], bf16, kind="ExternalOutput")

    def bcast(handle, n, offset=0):
        ap = handle[:]
        return bass.AP(tensor=ap.tensor, offset=offset, ap=[[0, P], [1, n]])

    with TileContext(nc) as tc:
        import contextlib
        ctx = contextlib.ExitStack()
        with ctx:
            const = ctx.enter_context(tc.tile_pool(name="const", bufs=1))
            posp = ctx.enter_context(tc.tile_pool(name="posp", bufs=2))
            xb_p = ctx.enter_context(tc.tile_pool(name="xb", bufs=2))
            xt_p = ctx.enter_context(tc.tile_pool(name="xt", bufs=2))
            qk_p = ctx.enter_context(tc.tile_pool(name="qk", bufs=2))
            v_p = ctx.enter_context(tc.tile_pool(name="vp", bufs=2))
            k2t_p = ctx.enter_context(tc.tile_pool(name="k2t", bufs=1))
            pb_p = ctx.enter_context(tc.tile_pool(name="pb", bufs=4))
            pt_p = ctx.enter_context(tc.tile_pool(name="pt", bufs=1))
            msk_p = ctx.enter_context(tc.tile_pool(name="msk", bufs=2))
            xpre_p = ctx.enter_context(tc.tile_pool(name="xpre", bufs=2))
            stat_p = ctx.enter_context(tc.tile_pool(name="stat", bufs=4))
            wts_p = ctx.enter_context(tc.tile_pool(name="wts", bufs=2))
            wp_p = ctx.enter_context(tc.tile_pool(name="wpp", bufs=2))
            bp_p = ctx.enter_context(tc.tile_pool(name="bpp", bufs=2))
            osb_p = ctx.enter_context(tc.tile_pool(name="osb", bufs=6))
            ps = ctx.enter_context(tc.tile_pool(name="ps", bufs=8, space="PSUM"))

            ident = const.tile([P, P], bf16)
            make_identity(nc, ident)
            trimask = const.tile([P, P], f32)
            nc.gpsimd.memset(trimask, 0.0)
            nc.gpsimd.affine_select(
                out=trimask, in_=trimask, compare_op=ALU.is_ge, fill=-1e10,
                base=0, pattern=[[-1, P]], channel_multiplier=1)

            # ---- embedding gather + positional encoding (critical path first) ----
            tok_sb = const.tile([P, ST], i32)
            nc.sync.dma_start(out=tok_sb,
                              in_=h_tok[:].rearrange("(a p) -> p a", p=P))
            xrows = xb_p.tile([P, ST, D], bf16, tag="xb", name="xrows")
            for a in range(ST):
                nc.gpsimd.indirect_dma_start(
                    out=xrows[:, a, :], out_offset=None, in_=h_table[:],
                    in_offset=bass.IndirectOffsetOnAxis(ap=tok_sb[:, a:a + 1],
                                                        axis=0))
            x0b = xb_p.tile([P, ST, D], bf16, tag="xb")
            for a in range(ST):
                post = posp.tile([P, D], f32, tag="pos")
                nc.sync.dma_start(out=post, in_=h_pos[a * P:(a + 1) * P, :])
                nc.vector.tensor_tensor(out=x0b[:, a, :], in0=xrows[:, a, :],
                                        in1=post, op=ALU.add)

            # ---- constants (order on the sync queue matters: small/early first) ----
            epst = const.tile([P, 1], f32)
            nc.vector.memset(epst, EPS)
            bq1s = const.tile([P, DT], f32)
            bk1s = const.tile([P, DT], f32)
            bq2s = const.tile([P, DT], f32)
            bk2s = const.tile([P, DT], f32)
            for t, h in ((bq1s, h_bq1), (bk1s, h_bk1), (bq2s, h_bq2),
                         (bk2s, h_bk2)):
                nc.sync.dma_start(out=t, in_=h[:])
            img_sb = const.tile([P, DIT, NI], bf16)
            nc.sync.dma_start(out=img_sb, in_=h_img[:])
            g1b = const.tile([P, D], f32)
            b1b = const.tile([P, D], f32)
            g2b = const.tile([P, D], f32)
            b2b = const.tile([P, D], f32)
            bv1b = const.tile([P, D], f32)
            bv2b = const.tile([P, D], f32)
            for t, h in ((g1b, h_g1), (b1b, h_b1), (g2b, h_g2), (b2b, h_b2),
                         (bv1b, h_bv1), (bv2b, h_bv2)):
                nc.sync.dma_start(out=t, in_=bcast(h, D))

            def transpose_x(xb_tile, tag):
                """[P, ST, D] bf16 (seq-partition) -> [P, DT, S] bf16 (d-partition)."""
                xt = xt_p.tile([P, DT, S], bf16, tag="xt", name=tag)
                for db in range(DT):
                    tp = ps.tile([P, 512], bf16, tag="ps", name="tp")
                    for a in range(ST):
                        nc.tensor.transpose(
                            out=tp[:, a * P:(a + 1) * P],
                            in_=xb_tile[:, a, db * P:(db + 1) * P],
                            identity=ident)
                        nc.vector.tensor_copy(
                            out=xt[:, db, a * P:(a + 1) * P],
                            in_=tp[:, a * P:(a + 1) * P])
                return xt

            x0T = transpose_x(x0b, "x0t")

            # ---- projections ----
            def proj_T(w_sb, b_sb, rhsT, name):
                """QT/KT-style: out[P, DT, S] bf16 = (W.T @ x.T) + b, d-partition."""
                o = qk_p.tile([P, DT, S], bf16, tag="qk", name=name)
                for m in range(DT):
                    pm = ps.tile([P, 512], f32, tag="ps", name="pm")
                    for k in range(DT):
                        nc.tensor.matmul(pm, lhsT=w_sb[:, k, m * P:(m + 1) * P],
                                         rhs=rhsT[:, k, :],
                                         start=(k == 0), stop=(k == DT - 1))
                    nc.scalar.activation(out=o[:, m, :], in_=pm,
                                         func=ACT_F.Identity,
                                         bias=b_sb[:, m:m + 1], scale=1.0)
                return o

            wq1_sb = wts_p.tile([P, DT, D], bf16, tag="wts")
            nc.scalar.dma_start(out=wq1_sb, in_=h_wq1[:])
            QT = proj_T(wq1_sb, bq1s, x0T, "qt")
            wk1_sb = wts_p.tile([P, DT, D], bf16, tag="wts")
            nc.sync.dma_start(out=wk1_sb, in_=h_wk1[:])
            KT = proj_T(wk1_sb, bk1s, x0T, "kt")

            wv1_sb = wts_p.tile([P, DT, D], bf16, tag="wts")
            nc.scalar.dma_start(out=wv1_sb, in_=h_wv1[:])
            Vt = v_p.tile([P, ST, D], bf16, tag="v")
            for a in range(ST):
                for nh in range(2):
                    pm = ps.tile([P, 512], f32, tag="ps")
                    for k in range(DT):
                        nc.tensor.matmul(
                            pm, lhsT=x0T[:, k, a * P:(a + 1) * P],
                            rhs=wv1_sb[:, k, nh * 512:(nh + 1) * 512],
                            start=(k == 0), stop=(k == DT - 1))
                    nc.vector.tensor_tensor(
                        out=Vt[:, a, nh * 512:(nh + 1) * 512], in0=pm,
                        in1=bv1b[:, nh * 512:(nh + 1) * 512], op=ALU.add)

            # ---- cross-attn K2/V2 (independent; fills PE while softmax runs) ----
            wk2_sb = wts_p.tile([P, DIT, D], bf16, tag="wts")
            nc.scalar.dma_start(out=wk2_sb, in_=h_wk2[:])
            K2T = k2t_p.tile([P, DT, NI_PAD], bf16, tag="k2t")
            for m in range(DT):
                pm = ps.tile([P, 512], f32, tag="ps")
                for k in range(DIT):
                    nc.tensor.matmul(pm[:, :NI],
                                     lhsT=wk2_sb[:, k, m * P:(m + 1) * P],
                                     rhs=img_sb[:, k, :],
                                     start=(k == 0), stop=(k == DIT - 1))
                nc.scalar.activation(out=K2T[:, m, :NI], in_=pm[:, :NI],
                                     func=ACT_F.Identity,
                                     bias=bk2s[:, m:m + 1], scale=1.0)

            wv2_sb = wts_p.tile([P, DIT, D], bf16, tag="wts")
            nc.sync.dma_start(out=wv2_sb, in_=h_wv2[:])
            V2t = v_p.tile([P, NIT, D], bf16, tag="v")
            nc.vector.memset(V2t, 0.0)
            for a in range(NIT):
                pa = P if a == 0 else NI - P
                for nh in range(2):
                    pm = ps.tile([P, 512], f32, tag="ps")
                    for k in range(DIT):
                        nc.tensor.matmul(
                            pm[:pa, :], lhsT=img_sb[:, k, a * P:a * P + pa],
                            rhs=wv2_sb[:, k, nh * 512:(nh + 1) * 512],
                            start=(k == 0), stop=(k == DIT - 1))
                    nc.vector.tensor_tensor(
                        out=V2t[:pa, a, nh * 512:(nh + 1) * 512], in0=pm[:pa, :],
                        in1=bv2b[:pa, nh * 512:(nh + 1) * 512], op=ALU.add)

            # ---- causal self-attention: scores + softmax (all qt), then AV ----
            Pbs = []
            rinv1 = stat_p.tile([P, ST], f32, tag="rinv")
            for qt in range(ST):
                width = (qt + 1) * P
                pm = ps.tile([P, 512], f32, tag="ps")
                for k in range(DT):
                    nc.tensor.matmul(pm[:, :width],
                                     lhsT=QT[:, k, qt * P:(qt + 1) * P],
                                     rhs=KT[:, k, :width],
                                     start=(k == 0), stop=(k == DT - 1))
                masked = msk_p.tile([P, 512], f32, tag="msk")
                if qt > 0:
                    nc.vector.tensor_copy(out=masked[:, :qt * P],
                                          in_=pm[:, :qt * P])
                nc.vector.tensor_tensor(out=masked[:, qt * P:width],
                                        in0=pm[:, qt * P:width], in1=trimask,
                                        op=ALU.add)
                nmax = stat_p.tile([P, 1], f32, tag="nmax")
                nc.vector.reduce_max(nmax, masked[:, :width], axis=X,
                                     negate=True)
                Pb = pb_p.tile([P, 512], bf16, tag="pb", name=f"pb{qt}")
                rsum = stat_p.tile([P, 1], f32, tag="rsum")
                nc.scalar.activation(out=Pb[:, :width], in_=masked[:, :width],
                                     func=ACT_F.Exp, bias=nmax, scale=1.0,
                                     accum_out=rsum)
                nc.vector.reciprocal(out=rinv1[:, qt:qt + 1], in_=rsum)
                Pbs.append(Pb)

            def layernorm(xpre, out_sl, gb, bb):
                """xpre [P, D] f32 -> out_sl [P, D] bf16 (normalized * g + b)."""
                stats = stat_p.tile([P, 2, 6], f32, tag="bnst")
                for sg in range(2):
                    nc.vector.bn_stats(out=stats[:, sg, :],
                                       in_=xpre[:, sg * 512:(sg + 1) * 512])
                mv = stat_p.tile([P, 2], f32, tag="bnmv")
                nc.vector.bn_aggr(out=mv, in_=stats)
                rstd = stat_p.tile([P, 1], f32, tag="rstd")
                nc.scalar.activation(out=rstd, in_=mv[:, 1:2], func=ACT_F.Sqrt,
                                     bias=epst, scale=1.0)
                nc.vector.reciprocal(out=rstd, in_=rstd)
                nmr = stat_p.tile([P, 1], f32, tag="nmr")
                nc.vector.tensor_tensor(out=nmr, in0=mv[:, 0:1], in1=rstd,
                                        op=ALU.mult)
                nc.scalar.mul(nmr, nmr, -1.0)
                nc.scalar.activation(out=xpre, in_=xpre, func=ACT_F.Identity,
                                     bias=nmr, scale=rstd)
                nc.vector.tensor_tensor(out=xpre, in0=xpre, in1=gb, op=ALU.mult)
                nc.vector.tensor_tensor(out=out_sl, in0=xpre, in1=bb, op=ALU.add)

            PT = pt_p.tile([P, ST, S], bf16, tag="pt")
            x1b = xb_p.tile([P, ST, D], bf16, tag="xb")
            for qt in range(ST):
                for kt in range(qt + 1):
                    tp = ps.tile([P, 512], bf16, tag="ps", name="tp")
                    nc.tensor.transpose(out=tp[:, :P],
                                        in_=Pbs[qt][:, kt * P:(kt + 1) * P],
                                        identity=ident)
                    nc.vector.tensor_copy(out=PT[:, kt, qt * P:(qt + 1) * P],
                                          in_=tp[:, :P])
                xpre = xpre_p.tile([P, D], f32, tag="xpre")
                for nh in range(2):
                    pm = ps.tile([P, 512], f32, tag="ps")
                    for kt in range(qt + 1):
                        nc.tensor.matmul(pm, lhsT=PT[:, kt, qt * P:(qt + 1) * P],
                                         rhs=Vt[:, kt, nh * 512:(nh + 1) * 512],
                                         start=(kt == 0), stop=(kt == qt))
                    nc.vector.scalar_tensor_tensor(
                        out=xpre[:, nh * 512:(nh + 1) * 512], in0=pm,
                        scalar=rinv1[:, qt:qt + 1],
                        in1=x0b[:, qt, nh * 512:(nh + 1) * 512],
                        op0=ALU.mult, op1=ALU.add)
                layernorm(xpre, x1b[:, qt, :], g1b, b1b)

            x1T = transpose_x(x1b, "x1t")

            # ---- cross attention: Q2, scores2 + softmax, then AV2 ----
            wq2_sb = wts_p.tile([P, DT, D], bf16, tag="wts")
            nc.scalar.dma_start(out=wq2_sb, in_=h_wq2[:])
            Q2T = proj_T(wq2_sb, bq2s, x1T, "q2t")

            P2bs = []
            rinv2 = stat_p.tile([P, ST], f32, tag="rinv")
            for qt in range(ST):
                pm = ps.tile([P, 512], f32, tag="ps")
                for k in range(DT):
                    nc.tensor.matmul(pm[:, :NI],
                                     lhsT=Q2T[:, k, qt * P:(qt + 1) * P],
                                     rhs=K2T[:, k, :NI],
                                     start=(k == 0), stop=(k == DT - 1))
                nmax = stat_p.tile([P, 1], f32, tag="nmax")
                nc.vector.reduce_max(nmax, pm[:, :NI], axis=X, negate=True)
                P2b = pb_p.tile([P, NI_PAD], bf16, tag="pb", name=f"p2b{qt}")
                nc.vector.memset(P2b[:, NI:], 0.0)
                rsum = stat_p.tile([P, 1], f32, tag="rsum")
                nc.scalar.activation(out=P2b[:, :NI], in_=pm[:, :NI],
                                     func=ACT_F.Exp, bias=nmax, scale=1.0,
                                     accum_out=rsum)
                nc.vector.reciprocal(out=rinv2[:, qt:qt + 1], in_=rsum)
                P2bs.append(P2b)

            PT2 = pt_p.tile([P, NIT, S], bf16, tag="pt")
            x2b = xb_p.tile([P, ST, D], bf16, tag="xb")
            for qt in range(ST):
                for kt in range(NIT):
                    tp = ps.tile([P, 512], bf16, tag="ps", name="tp")
                    nc.tensor.transpose(out=tp[:, :P],
                                        in_=P2bs[qt][:, kt * P:(kt + 1) * P],
                                        identity=ident)
                    nc.vector.tensor_copy(out=PT2[:, kt, qt * P:(qt + 1) * P],
                                          in_=tp[:, :P])
                xpre = xpre_p.tile([P, D], f32, tag="xpre")
                for nh in range(2):
                    pm = ps.tile([P, 512], f32, tag="ps")
                    for kt in range(NIT):
                        nc.tensor.matmul(pm, lhsT=PT2[:, kt, qt * P:(qt + 1) * P],
                                         rhs=V2t[:, kt, nh * 512:(nh + 1) * 512],
                                         start=(kt == 0), stop=(kt == NIT - 1))
                    nc.vector.scalar_tensor_tensor(
                        out=xpre[:, nh * 512:(nh + 1) * 512], in0=pm,
                        scalar=rinv2[:, qt:qt + 1],
                        in1=x1b[:, qt, nh * 512:(nh + 1) * 512],
                        op0=ALU.mult, op1=ALU.add)
                layernorm(xpre, x2b[:, qt, :], g2b, b2b)

            x2T = transpose_x(x2b, "x2t")

            # ---- vocab projection, streamed in CN-column chunks ----
            for g in range(NGRP):
                bp_bc = bp_p.tile([P, GRP * CN], bf16, tag="bp")
                nc.scalar.dma_start(out=bp_bc,
                                    in_=bcast(h_bp, GRP * CN,
                                              offset=g * GRP * CN))
                osb = [osb_p.tile([P, GRP * CN], bf16, tag="osb",
                                  name=f"osb_{g}_{q}")
                       for q in range(ST)]
                for cc in range(GRP):
                    c = g * GRP + cc
                    wp_sb = wp_p.tile([P, DT, CN], bf16, tag="wp")
                    dma_eng = nc.sync if c % 2 == 0 else nc.scalar
                    dma_eng.dma_start(out=wp_sb, in_=h_wp[c])
                    for qt in range(ST):
                        for nh in range(2):
                            pm = ps.tile([P, 512], f32, tag="ps")
                            for k in range(DT):
                                nc.tensor.matmul(
                                    pm, lhsT=x2T[:, k, qt * P:(qt + 1) * P],
                                    rhs=wp_sb[:, k, nh * 512:(nh + 1) * 512],
                                    start=(k == 0), stop=(k == DT - 1))
                            nc.vector.tensor_tensor(
                                out=osb[qt][:, (cc * 2 + nh) * 512:
                                            (cc * 2 + nh + 1) * 512], in0=pm,
                                in1=bp_bc[:, (cc * 2 + nh) * 512:
                                          (cc * 2 + nh + 1) * 512], op=ALU.add)
                for qt in range(ST):
                    out_eng = nc.sync if qt < 2 else nc.scalar
                    out_eng.dma_start(
                        out=h_out[qt * P:(qt + 1) * P,
                                  g * GRP * CN:(g + 1) * GRP * CN],
                        in_=osb[qt])

    nc.compile()
    return nc


def _tile_sq(w, kt):
    """[K, N] -> [128, K//128, N] contiguous."""
    k, n = w.shape
    assert k == kt * P
    return np.ascontiguousarray(
        w.reshape(kt, P, n).transpose(1, 0, 2)).astype(BF16)


def _prep_inputs(inputs):
    g = lambda name: np.asarray(inputs[name], dtype=np.float32)
    tokens = np.asarray(inputs["tokens"]).astype(np.int32)
    img = g("img_emb")

    # positional encoding (same closed form as the model definition)
    posn = np.arange(S)[:, None].astype(np.float32)
    i = np.arange(0, D, 2).astype(np.float32)
    ang = posn / np.power(10000.0, i / D)
    pos = np.zeros((S, D), dtype=np.float32)
    pos[:, 0::2] = np.sin(ang)
    pos[:, 1::2] = np.cos(ang)

    wp = g("Wp")  # [D, V]
    wp_pad = np.zeros((D, VP), dtype=np.float32)
    wp_pad[:, :V] = wp
    wp_t = np.ascontiguousarray(
        wp_pad.reshape(DT, P, NCHUNK, CN).transpose(2, 1, 0, 3)).astype(BF16)
    bp_pad = np.zeros((VP,), dtype=np.float32)
    bp_pad[:V] = g("bp")
    bp_pad = bp_pad.astype(BF16)

    def bias_tiled(b):
        return np.ascontiguousarray(b.reshape(DT, P).T).astype(np.float32)

    shared = {
        "table": g("emb_table").astype(BF16),
        "pos": pos,
        "wq1": _tile_sq(g("Wq1") * SCALE, DT),
        "wk1": _tile_sq(g("Wk1"), DT),
        "wv1": _tile_sq(g("Wv1"), DT),
        "wq2": _tile_sq(g("Wq2") * SCALE, DT),
        "wk2": _tile_sq(g("Wk2"), DIT),
        "wv2": _tile_sq(g("Wv2"), DIT),
        "wp": wp_t,
        "bq1": bias_tiled(g("bq1") * SCALE),
        "bk1": bias_tiled(g("bk1")),
        "bq2": bias_tiled(g("bq2") * SCALE),
        "bk2": bias_tiled(g("bk2")),
        "bv1": g("bv1"),
        "bv2": g("bv2"),
        "bp": bp_pad,
        "g1": g("g1"), "b1": g("b1"), "g2": g("g2"), "b2": g("b2"),
    }
    in_maps = []
    for c in range(N_CORES):
        m = dict(shared)
        m["tok"] = np.ascontiguousarray(tokens[c])
        m["img_t"] = np.ascontiguousarray(
            img[c].T.reshape(DIT, P, NI).transpose(1, 0, 2)).astype(BF16)
        in_maps.append(m)
    return in_maps


def kernel(**inputs):
    global LAST_RESULTS
    from concourse.bass_utils import run_bass_kernel_spmd

    if "nc" not in _CACHE:
        _CACHE["nc"] = _build_program()
    nc = _CACHE["nc"]

    in_maps = _prep_inputs(inputs)
    res = run_bass_kernel_spmd(nc, in_maps, core_ids=list(range(N_CORES)))
    LAST_RESULTS = res
    out = np.stack([res.results[c]["out"][:, :V].astype(np.float32)
                    for c in range(N_CORES)])
    return out


# revision 24
# speedup vs baseline: 1.0004x; 1.0004x over previous
"""Trainium2 Bass kernel for an 8-batch image-conditioned decoder layer.

Strategy: pure data-parallel over the batch — core c computes batch element c
end-to-end (embedding gather, causal self-attention, cross-attention over the
image tokens, both layernorms, vocab projection). No collectives.

All matmuls run in bf16 with fp32 PSUM accumulation.  Weights are pre-cast /
pre-tiled on the host into the exact SBUF layouts the TensorEngine consumes
([128 k_inner, k_outer, n]); the vocab projection is streamed from HBM in
512-column chunks.  The embedding gather happens on-device via indirect DMA
from a bf16 copy of the table.
"""

import os
import sys

for _p in ("/opt/trn_rl_repo", "/root/.axon_site/_ro/trn_rl_repo"):
    if os.path.isdir(_p) and _p not in sys.path:
        sys.path.append(_p)

import numpy as np
import ml_dtypes

BF16 = ml_dtypes.bfloat16

# Problem dims (hardcoded per spec)
V, D, DI, S, B, NI = 32000, 1024, 768, 512, 8, 197
EPS = 1e-5
P = 128
ST = S // P          # 4 seq tiles
DT = D // P          # 8 model-dim tiles
DIT = DI // P        # 6 image-dim tiles
NIT = 2              # image tokens: 197 -> 2 partition tiles (128 + 69)
NI_PAD = 256
VP = 32768           # vocab padded to 64 chunks of 512
CN = 512             # vocab chunk width
NCHUNK = VP // CN    # 64
GRP = 2              # chunks per output strip
NGRP = NCHUNK // GRP
N_CORES = 8
SCALE = 1.0 / float(np.sqrt(np.float32(D)))

_CACHE = {}
LAST_RESULTS = None


def _build_program():
    import concourse.bacc as bacc
    import concourse.bass as bass
    import concourse.mybir as mybir
    from concourse.masks import make_identity
    from concourse.tile import TileContext

    f32 = mybir.dt.float32
    bf16 = mybir.dt.bfloat16
    i32 = mybir.dt.int32
    X = mybir.AxisListType.X
    ALU = mybir.AluOpType
    ACT_F = mybir.ActivationFunctionType

    nc = bacc.Bacc("TRN2", target_bir_lowering=False, debug=False,
                   num_devices=N_CORES)

    # ---- I/O ----
    h_tok = nc.dram_tensor("tok", [S], i32, kind="ExternalInput")
    h_table = nc.dram_tensor("table", [V, D], bf16, kind="ExternalInput")
    h_pos = nc.dram_tensor("pos", [S, D], f32, kind="ExternalInput")
    h_img = nc.dram_tensor("img_t", [P, DIT, NI], bf16, kind="ExternalInput")
    h_wq1 = nc.dram_tensor("wq1", [P, DT, D], bf16, kind="ExternalInput")
    h_wk1 = nc.dram_tensor("wk1", [P, DT, D], bf16, kind="ExternalInput")
    h_wv1 = nc.dram_tensor("wv1", [P, DT, D], bf16, kind="ExternalInput")
    h_wq2 = nc.dram_tensor("wq2", [P, DT, D], bf16, kind="ExternalInput")
    h_wk2 = nc.dram_tensor("wk2", [P, DIT, D], bf16, kind="ExternalInput")
    h_wv2 = nc.dram_tensor("wv2", [P, DIT, D], bf16, kind="ExternalInput")
    h_wp = nc.dram_tensor("wp", [NCHUNK, P, DT, CN], bf16, kind="ExternalInput")
    h_bq1 = nc.dram_tensor("bq1", [P, DT], f32, kind="ExternalInput")
    h_bk1 = nc.dram_tensor("bk1", [P, DT], f32, kind="ExternalInput")
    h_bq2 = nc.dram_tensor("bq2", [P, DT], f32, kind="ExternalInput")
    h_bk2 = nc.dram_tensor("bk2", [P, DT], f32, kind="ExternalInput")
    h_bv1 = nc.dram_tensor("bv1", [D], f32, kind="ExternalInput")
    h_bv2 = nc.dram_tensor("bv2", [D], f32, kind="ExternalInput")
    h_bp = nc.dram_tensor("bp", [VP], bf16, kind="ExternalInput")
    h_g1 = nc.dram_tensor("g1", [D], f32, kind="ExternalInput")
    h_b1 = nc.dram_tensor("b1", [D], f32, kind="ExternalInput")
    h_g2 = nc.dram_tensor("g2", [D], f32, kind="ExternalInput")
    h_b2 = nc.dram_tensor("b2", [D], f32, kind="ExternalInput")
    h_out = nc.dram_tensor("out", [S, VP], bf16, kind="ExternalOutput")

    def bcast(handle, n, offset=0):
        ap = handle[:]
        return bass.AP(tensor=ap.tensor, offset=offset, ap=[[0, P], [1, n]])

    with TileContext(nc) as tc:
        import contextlib
        ctx = contextlib.ExitStack()
        with ctx:
            const = ctx.enter_context(tc.tile_pool(name="const", bufs=1))
            posp = ctx.enter_context(tc.tile_pool(name="posp", bufs=2))
            xb_p = ctx.enter_context(tc.tile_pool(name="xb", bufs=2))
            xt_p = ctx.enter_context(tc.tile_pool(name="xt", bufs=2))
            qk_p = ctx.enter_context(tc.tile_pool(name="qk", bufs=2))
            v_p = ctx.enter_context(tc.tile_pool(name="vp", bufs=2))
            k2t_p = ctx.enter_context(tc.tile_pool(name="k2t", bufs=1))
            pb_p = ctx.enter_context(tc.tile_pool(name="pb", bufs=4))
            pt_p = ctx.enter_context(tc.tile_pool(name="pt", bufs=1))
            msk_p = ctx.enter_context(tc.tile_pool(name="msk", bufs=2))
            xpre_p = ctx.enter_context(tc.tile_pool(name="xpre", bufs=2))
            stat_p = ctx.enter_context(tc.tile_pool(name="stat", bufs=4))
            wts_p = ctx.enter_context(tc.tile_pool(name="wts", bufs=2))
            wp_p = ctx.enter_context(tc.tile_pool(name="wpp", bufs=4))
            bp_p = ctx.enter_context(tc.tile_pool(name="bpp", bufs=2))
            osb_p = ctx.enter_context(tc.tile_pool(name="osb", bufs=6))
            ps = ctx.enter_context(tc.tile_pool(name="ps", bufs=8, space="PSUM"))

            ident = const.tile([P, P], bf16)
            make_identity(nc, ident)
            trimask = const.tile([P, P], f32)
            nc.gpsimd.memset(trimask, 0.0)
            nc.gpsimd.affine_select(
                out=trimask, in_=trimask, compare_op=ALU.is_ge, fill=-1e10,
                base=0, pattern=[[-1, P]], channel_multiplier=1)

            # ---- embedding gather + positional encoding (critical path first) ----
            tok_sb = const.tile([P, ST], i32)
            nc.sync.dma_start(out=tok_sb,
                              in_=h_tok[:].rearrange("(a p) -> p a", p=P))
            xrows = xb_p.tile([P, ST, D], bf16, tag="xb", name="xrows")
            for a in range(ST):
                nc.gpsimd.indirect_dma_start(
                    out=xrows[:, a, :], out_offset=None, in_=h_table[:],
                    in_offset=bass.IndirectOffsetOnAxis(ap=tok_sb[:, a:a + 1],
                                                        axis=0))
            x0b = xb_p.tile([P, ST, D], bf16, tag="xb")
            for a in range(ST):
                post = posp.tile([P, D], f32, tag="pos")
                nc.sync.dma_start(out=post, in_=h_pos[a * P:(a + 1) * P, :])
                nc.vector.tensor_tensor(out=x0b[:, a, :], in0=xrows[:, a, :],
                                        in1=post, op=ALU.add)

            # ---- constants (order on the sync queue matters: small/early first) ----
            epst = const.tile([P, 1], f32)
            nc.vector.memset(epst, EPS)
            bq1s = const.tile([P, DT], f32)
            bk1s = const.tile([P, DT], f32)
            bq2s = const.tile([P, DT], f32)
            bk2s = const.tile([P, DT], f32)
            for t, h in ((bq1s, h_bq1), (bk1s, h_bk1), (bq2s, h_bq2),
                         (bk2s, h_bk2)):
                nc.sync.dma_start(out=t, in_=h[:])
            img_sb = const.tile([P, DIT, NI], bf16)
            nc.sync.dma_start(out=img_sb, in_=h_img[:])
            g1b = const.tile([P, D], f32)
            b1b = const.tile([P, D], f32)
            g2b = const.tile([P, D], f32)
            b2b = const.tile([P, D], f32)
            bv1b = const.tile([P, D], f32)
            bv2b = const.tile([P, D], f32)
            for t, h in ((g1b, h_g1), (b1b, h_b1), (g2b, h_g2), (b2b, h_b2),
                         (bv1b, h_bv1), (bv2b, h_bv2)):
                nc.sync.dma_start(out=t, in_=bcast(h, D))

            def transpose_x(xb_tile, tag):
                """[P, ST, D] bf16 (seq-partition) -> [P, DT, S] bf16 (d-partition)."""
                xt = xt_p.tile([P, DT, S], bf16, tag="xt", name=tag)
                for db in range(DT):
                    tp = ps.tile([P, 512], bf16, tag="ps", name="tp")
                    for a in range(ST):
                        nc.tensor.transpose(
                            out=tp[:, a * P:(a + 1) * P],
                            in_=xb_tile[:, a, db * P:(db + 1) * P],
                            identity=ident)
                        nc.vector.tensor_copy(
                            out=xt[:, db, a * P:(a + 1) * P],
                            in_=tp[:, a * P:(a + 1) * P])
                return xt

            x0T = transpose_x(x0b, "x0t")

            # ---- projections ----
            def proj_T(w_sb, b_sb, rhsT, name):
                """QT/KT-style: out[P, DT, S] bf16 = (W.T @ x.T) + b, d-partition."""
                o = qk_p.tile([P, DT, S], bf16, tag="qk", name=name)
                for m in range(DT):
                    pm = ps.tile([P, 512], f32, tag="ps", name="pm")
                    for k in range(DT):
                        nc.tensor.matmul(pm, lhsT=w_sb[:, k, m * P:(m + 1) * P],
                                         rhs=rhsT[:, k, :],
                                         start=(k == 0), stop=(k == DT - 1))
                    nc.scalar.activation(out=o[:, m, :], in_=pm,
                                         func=ACT_F.Identity,
                                         bias=b_sb[:, m:m + 1], scale=1.0)
                return o

            wq1_sb = wts_p.tile([P, DT, D], bf16, tag="wts")
            nc.scalar.dma_start(out=wq1_sb, in_=h_wq1[:])
            QT = proj_T(wq1_sb, bq1s, x0T, "qt")
            wk1_sb = wts_p.tile([P, DT, D], bf16, tag="wts")
            nc.sync.dma_start(out=wk1_sb, in_=h_wk1[:])
            KT = proj_T(wk1_sb, bk1s, x0T, "kt")

            wv1_sb = wts_p.tile([P, DT, D], bf16, tag="wts")
            nc.scalar.dma_start(out=wv1_sb, in_=h_wv1[:])
            Vt = v_p.tile([P, ST, D], bf16, tag="v")
            for a in range(ST):
                for nh in range(2):
                    pm = ps.tile([P, 512], f32, tag="ps")
                    for k in range(DT):
                        nc.tensor.matmul(
                            pm, lhsT=x0T[:, k, a * P:(a + 1) * P],
                            rhs=wv1_sb[:, k, nh * 512:(nh + 1) * 512],
                            start=(k == 0), stop=(k == DT - 1))
                    nc.vector.tensor_tensor(
                        out=Vt[:, a, nh * 512:(nh + 1) * 512], in0=pm,
                        in1=bv1b[:, nh * 512:(nh + 1) * 512], op=ALU.add)

            # ---- cross-attn K2/V2 (independent; fills PE while softmax runs) ----
            wk2_sb = wts_p.tile([P, DIT, D], bf16, tag="wts")
            nc.scalar.dma_start(out=wk2_sb, in_=h_wk2[:])
            K2T = k2t_p.tile([P, DT, NI_PAD], bf16, tag="k2t")
            for m in range(DT):
                pm = ps.tile([P, 512], f32, tag="ps")
                for k in range(DIT):
                    nc.tensor.matmul(pm[:, :NI],
                                     lhsT=wk2_sb[:, k, m * P:(m + 1) * P],
                                     rhs=img_sb[:, k, :],
                                     start=(k == 0), stop=(k == DIT - 1))
                nc.scalar.activation(out=K2T[:, m, :NI], in_=pm[:, :NI],
                                     func=ACT_F.Identity,
                                     bias=bk2s[:, m:m + 1], scale=1.0)

            wv2_sb = wts_p.tile([P, DIT, D], bf16, tag="wts")
            nc.sync.dma_start(out=wv2_sb, in_=h_wv2[:])
            V2t = v_p.tile([P, NIT, D], bf16, tag="v")
            nc.vector.memset(V2t, 0.0)
            for a in range(NIT):
                pa = P if a == 0 else NI - P
                for nh in range(2):
                    pm = ps.tile([P, 512], f32, tag="ps")
                    for k in range(DIT):
                        nc.tensor.matmul(
                            pm[:pa, :], lhsT=img_sb[:, k, a * P:a * P + pa],
                            rhs=wv2_sb[:, k, nh * 512:(nh + 1) * 512],
                            start=(k == 0), stop=(k == DIT - 1))
                    nc.vector.tensor_tensor(
                        out=V2t[:pa, a, nh * 512:(nh + 1) * 512], in0=pm[:pa, :],
                        in1=bv2b[:pa, nh * 512:(nh + 1) * 512], op=ALU.add)

            # ---- causal self-attention: scores + softmax (all qt), then AV ----
            Pbs = []
            rinv1 = stat_p.tile([P, ST], f32, tag="rinv")
            for qt in range(ST):
                width = (qt + 1) * P
                pm = ps.tile([P, 512], f32, tag="ps")
                for k in range(DT):
                    nc.tensor.matmul(pm[:, :width],
                                     lhsT=QT[:, k, qt * P:(qt + 1) * P],
                                     rhs=KT[:, k, :width],
                                     start=(k == 0), stop=(k == DT - 1))
                masked = msk_p.tile([P, 512], f32, tag="msk")
                if qt > 0:
                    nc.vector.tensor_copy(out=masked[:, :qt * P],
                                          in_=pm[:, :qt * P])
                nc.vector.tensor_tensor(out=masked[:, qt * P:width],
                                        in0=pm[:, qt * P:width], in1=trimask,
                                        op=ALU.add)
                nmax = stat_p.tile([P, 1], f32, tag="nmax")
                nc.vector.reduce_max(nmax, masked[:, :width], axis=X,
                                     negate=True)
                Pb = pb_p.tile([P, 512], bf16, tag="pb", name=f"pb{qt}")
                rsum = stat_p.tile([P, 1], f32, tag="rsum")
                nc.scalar.activation(out=Pb[:, :width], in_=masked[:, :width],
                                     func=ACT_F.Exp, bias=nmax, scale=1.0,
                                     accum_out=rsum)
                nc.vector.reciprocal(out=rinv1[:, qt:qt + 1], in_=rsum)
                Pbs.append(Pb)

            def layernorm(xpre, out_sl, gb, bb):
                """xpre [P, D] f32 -> out_sl [P, D] bf16 (normalized * g + b)."""
                stats = stat_p.tile([P, 2, 6], f32, tag="bnst")
                for sg in range(2):
                    nc.vector.bn_stats(out=stats[:, sg, :],
                                       in_=xpre[:, sg * 512:(sg + 1) * 512])
                mv = stat_p.tile([P, 2], f32, tag="bnmv")
                nc.vector.bn_aggr(out=mv, in_=stats)
                rstd = stat_p.tile([P, 1], f32, tag="rstd")
                nc.scalar.activation(out=rstd, in_=mv[:, 1:2], func=ACT_F.Sqrt,
                                     bias=epst, scale=1.0)
                nc.vector.reciprocal(out=rstd, in_=rstd)
                nmr = stat_p.tile([P, 1], f32, tag="nmr")
                nc.vector.tensor_tensor(out=nmr, in0=mv[:, 0:1], in1=rstd,
                                        op=ALU.mult)
                nc.scalar.mul(nmr, nmr, -1.0)
                nc.scalar.activation(out=xpre, in_=xpre, func=ACT_F.Identity,
                                     bias=nmr, scale=rstd)
                nc.vector.tensor_tensor(out=xpre, in0=xpre, in1=gb, op=ALU.mult)
                nc.vector.tensor_tensor(out=out_sl, in0=xpre, in1=bb, op=ALU.add)

            PT = pt_p.tile([P, ST, S], bf16, tag="pt")
            x1b = xb_p.tile([P, ST, D], bf16, tag="xb")
            for qt in range(ST):
                for kt in range(qt + 1):
                    tp = ps.tile([P, 512], bf16, tag="ps", name="tp")
                    nc.tensor.transpose(out=tp[:, :P],
                                        in_=Pbs[qt][:, kt * P:(kt + 1) * P],
                                        identity=ident)
                    nc.vector.tensor_copy(out=PT[:, kt, qt * P:(qt + 1) * P],
                                          in_=tp[:, :P])
                xpre = xpre_p.tile([P, D], f32, tag="xpre")
                for nh in range(2):
                    pm = ps.tile([P, 512], f32, tag="ps")
                    for kt in range(qt + 1):
                        nc.tensor.matmul(pm, lhsT=PT[:, kt, qt * P:(qt + 1) * P],
                                         rhs=Vt[:, kt, nh * 512:(nh + 1) * 512],
                                         start=(kt == 0), stop=(kt == qt))
                    nc.vector.scalar_tensor_tensor(
                        out=xpre[:, nh * 512:(nh + 1) * 512], in0=pm,
                        scalar=rinv1[:, qt:qt + 1],
                        in1=x0b[:, qt, nh * 512:(nh + 1) * 512],
                        op0=ALU.mult, op1=ALU.add)
                layernorm(xpre, x1b[:, qt, :], g1b, b1b)

            x1T = transpose_x(x1b, "x1t")

            # ---- cross attention: Q2, scores2 + softmax, then AV2 ----
            wq2_sb = wts_p.tile([P, DT, D], bf16, tag="wts")
            nc.scalar.dma_start(out=wq2_sb, in_=h_wq2[:])
            Q2T = proj_T(wq2_sb, bq2s, x1T, "q2t")

            P2bs = []
            rinv2 = stat_p.tile([P, ST], f32, tag="rinv")
            for qt in range(ST):
                pm = ps.tile([P, 512], f32, tag="ps")
                for k in range(DT):
                    nc.tensor.matmul(pm[:, :NI],
                                     lhsT=Q2T[:, k, qt * P:(qt + 1) * P],
                                     rhs=K2T[:, k, :NI],
                                     start=(k == 0), stop=(k == DT - 1))
                nmax = stat_p.tile([P, 1], f32, tag="nmax")
                nc.vector.reduce_max(nmax, pm[:, :NI], axis=X, negate=True)
                P2b = pb_p.tile([P, NI_PAD], bf16, tag="pb", name=f"p2b{qt}")
                nc.vector.memset(P2b[:, NI:], 0.0)
                rsum = stat_p.tile([P, 1], f32, tag="rsum")
                nc.scalar.activation(out=P2b[:, :NI], in_=pm[:, :NI],
                                     func=ACT_F.Exp, bias=nmax, scale=1.0,
                                     accum_out=rsum)
                nc.vector.reciprocal(out=rinv2[:, qt:qt + 1], in_=rsum)
                P2bs.append(P2b)

            PT2 = pt_p.tile([P, NIT, S], bf16, tag="pt")
            x2b = xb_p.tile([P, ST, D], bf16, tag="xb")
            for qt in range(ST):
                for kt in range(NIT):
                    tp = ps.tile([P, 512], bf16, tag="ps", name="tp")
                    nc.tensor.transpose(out=tp[:, :P],
                                        in_=P2bs[qt][:, kt * P:(kt + 1) * P],
                                        identity=ident)
                    nc.vector.tensor_copy(out=PT2[:, kt, qt * P:(qt + 1) * P],
                                          in_=tp[:, :P])
                xpre = xpre_p.tile([P, D], f32, tag="xpre")
                for nh in range(2):
                    pm = ps.tile([P, 512], f32, tag="ps")
                    for kt in range(NIT):
                        nc.tensor.matmul(pm, lhsT=PT2[:, kt, qt * P:(qt + 1) * P],
                                         rhs=V2t[:, kt, nh * 512:(nh + 1) * 512],
                                         start=(kt == 0), stop=(kt == NIT - 1))
                    nc.vector.scalar_tensor_tensor(
                        out=xpre[:, nh * 512:(nh + 1) * 512], in0=pm,
                        scalar=rinv2[:, qt:qt + 1],
                        in1=x1b[:, qt, nh * 512:(nh + 1) * 512],
                        op0=ALU.mult, op1=ALU.add)
                layernorm(xpre, x2b[:, qt, :], g2b, b2b)

            x2T = transpose_x(x2b, "x2t")

            # ---- vocab projection, streamed in CN-column chunks ----
            for g in range(NGRP):
                bp_bc = bp_p.tile([P, GRP * CN], bf16, tag="bp")
                nc.scalar.dma_start(out=bp_bc,
                                    in_=bcast(h_bp, GRP * CN,
                                              offset=g * GRP * CN))
                osb = [osb_p.tile([P, GRP * CN], bf16, tag="osb",
                                  name=f"osb_{g}_{q}")
                       for q in range(ST)]
                for cc in range(GRP):
                    c = g * GRP + cc
                    wp_sb = wp_p.tile([P, DT, CN], bf16, tag="wp")
                    dma_eng = nc.sync if c % 2 == 0 else nc.scalar
                    dma_eng.dma_start(out=wp_sb, in_=h_wp[c])
                    for qt in range(ST):
                        pm = ps.tile([P, 512], f32, tag="ps")
                        for k in range(DT):
                            nc.tensor.matmul(
                                pm, lhsT=x2T[:, k, qt * P:(qt + 1) * P],
                                rhs=wp_sb[:, k, :],
                                start=(k == 0), stop=(k == DT - 1))
                        nc.vector.tensor_tensor(
                            out=osb[qt][:, cc * CN:(cc + 1) * CN], in0=pm,
                            in1=bp_bc[:, cc * CN:(cc + 1) * CN], op=ALU.add)
                for qt in range(ST):
                    out_eng = nc.sync if qt < 2 else nc.scalar
                    out_eng.dma_start(
                        out=h_out[qt * P:(qt + 1) * P,
                                  g * GRP * CN:(g + 1) * GRP * CN],
                        in_=osb[qt])

    nc.compile()
    return nc


def _tile_sq(w, kt):
    """[K, N] -> [128, K//128, N] contiguous."""
    k, n = w.shape
    assert k == kt * P
    return np.ascontiguousarray(
        w.reshape(kt, P, n).transpose(1, 0, 2)).astype(BF16)


def _prep_inputs(inputs):
    g = lambda name: np.asarray(inputs[name], dtype=np.float32)
    tokens = np.asarray(inputs["tokens"]).astype(np.int32)
    img = g("img_emb")

    # positional encoding (same closed form as the model definition)
    posn = np.arange(S)[:, None].astype(np.float32)
    i = np.arange(0, D, 2).astype(np.float32)
    ang = posn / np.power(10000.0, i / D)
    pos = np.zeros((S, D), dtype=np.float32)
    pos[:, 0::2] = np.sin(ang)
    pos[:, 1::2] = np.cos(ang)

    wp = g("Wp")  # [D, V]
    wp_pad = np.zeros((D, VP), dtype=np.float32)
    wp_pad[:, :V] = wp
    wp_t = np.ascontiguousarray(
        wp_pad.reshape(DT, P, NCHUNK, CN).transpose(2, 1, 0, 3)).astype(BF16)
    bp_pad = np.zeros((VP,), dtype=np.float32)
    bp_pad[:V] = g("bp")
    bp_pad = bp_pad.astype(BF16)

    def bias_tiled(b):
        return np.ascontiguousarray(b.reshape(DT, P).T).astype(np.float32)

    shared = {
        "table": g("emb_table").astype(BF16),
        "pos": pos,
        "wq1": _tile_sq(g("Wq1") * SCALE, DT),
        "wk1": _tile_sq(g("Wk1"), DT),
        "wv1": _tile_sq(g("Wv1"), DT),
        "wq2": _tile_sq(g("Wq2") * SCALE, DT),
        "wk2": _tile_sq(g("Wk2"), DIT),
        "wv2": _tile_sq(g("Wv2"), DIT),
        "wp": wp_t,
        "bq1": bias_tiled(g("bq1") * SCALE),
        "bk1": bias_tiled(g("bk1")),
        "bq2": bias_tiled(g("bq2") * SCALE),
        "bk2": bias_tiled(g("bk2")),
        "bv1": g("bv1"),
        "bv2": g("bv2"),
        "bp": bp_pad,
        "g1": g("g1"), "b1": g("b1"), "g2": g("g2"), "b2": g("b2"),
    }
    in_maps = []
    for c in range(N_CORES):
        m = dict(shared)
        m["tok"] = np.ascontiguousarray(tokens[c])
        m["img_t"] = np.ascontiguousarray(
            img[c].T.reshape(DIT, P, NI).transpose(1, 0, 2)).astype(BF16)
        in_maps.append(m)
    return in_maps


def kernel(**inputs):
    global LAST_RESULTS
    from concourse.bass_utils import run_bass_kernel_spmd

    if "nc" not in _CACHE:
        _CACHE["nc"] = _build_program()
    nc = _CACHE["nc"]

    in_maps = _prep_inputs(inputs)
    res = run_bass_kernel_spmd(nc, in_maps, core_ids=list(range(N_CORES)))
    LAST_RESULTS = res
    out = np.stack([res.results[c]["out"][:, :V].astype(np.float32)
                    for c in range(N_CORES)])
    return out
